# revision 33
# baseline (speedup 1.0000x reference)
"""Trainium2 Bass kernel for the 50-step autoregressive MLP rollout.

Per batch row b: state = x[b,0,2:9]; for t: h1 = tanh([u_t, s] @ W1);
h2 = tanh(h1 @ W2); s += DT * (h2 @ W3); out[b,t] = s.

Pure data parallel: 4096 rows/core on 8 cores; no collectives.  The
cost-model timeline is ~759us/core vs 955us for the f32r baseline; the
Activation engine (the bottleneck: 2x256 tanh per row-step at 1 elem/
lane/cycle) runs at ~96% occupancy.

Key design points:
- Feature-major on chip.  Batch in 8 chunks of 512; chunk c = (h, q),
  half h = c//4, quadrant q = c%4.
- The integrator state lives in PSUM for the whole scan as z = state/DT
  ([103, 512] f32 per half; quadrant q's 7 state rows at partitions
  32q..32q+6).  Layer 3 (bf16) accumulates into z with start=False.
  The 1/DT scaling keeps W3 at its natural scale (DT*W3 would sit in
  fp8-e4m3's denormal range; it also lets fp8 be considered for W3).
- One DVE tensor_scalar per half per step copies z*DT -> bf16 sT tile,
  which doubles as layer-1 moving operand and as the DMA'd output.
- Layer 1: controls via fp8 DoubleRow (stationary holds W1u/2 twice,
  moving broadcast stride-0 - full DR rate with K=2) + state in bf16,
  accumulated per quadrant via tile_position.
- Layer 2: fp8 DoubleRow with an fp8 *residual* correction (W2 ~=
  q8(W2) + q8(W2 - q8(W2)), two DR matmuls) - fp8 speed, ~bf16-class
  systematic error.  DoubleRow needs M>=32 and column position 0,
  which is why layer 3 (M=7, quadrant positions) stays bf16.
- Two chunks per step bypass the Activation engine for the h1 tanh:
  a DVE chain computes the Pade(3,2) approximant x(15+x^2)/(15+6x^2)
  (max err ~5e-3 on the observed |x|<=2.9 range) via tensor ops and
  reciprocal_approx_fast; their layer 2 runs bf16 on a separate W2
  copy.  This offloads ~12% of the activation-engine work.
- All PSUM psum tiles for h1/h2 preactivations come from one shared
  3-deep [128,1024] ring (6 banks) + 2 z banks = 8 banks; the 3-deep
  rotation hides the write-after-read semaphore loop that a dedicated
  single ph2 buffer would serialize on.
- stage2 (L2+tanh+L3) trails stage1 (L1+tanh) by >=2 chunks, stage2
  is emitted before stage1 within an iteration, and the s1o/s2o chunk
  orders interleave the Pade chunks so every engine handoff has >=1
  activation-slot of slack (schedules found empirically against the
  cost-model timeline).
- Host does all packing/unpacking (transposes, dtype casts, fp8/bf16
  quantization, output reshape); only the scan runs on device.  Biases
  are zeros per the spec and asserted so.
- End-to-end rel err vs the jax reference: 4.9e-3 (tolerance 2e-2).
"""

import numpy as np

B_TOTAL = 32768
N_CORES = 8
B_CORE = B_TOTAL // N_CORES      # 4096
H = 50
F = 9
NCTRL = 2
NST = 7
HID = 256
DT = 0.02
NT = 512                         # chunk batch size
NCH = B_CORE // NT               # 8 chunks

_CACHE = {}


def _build(horizon=H, pade=(4, 5), s1o=(0, 1, 2, 4, 3, 5, 6, 7),
           s2o=(0, 1, 2, 6, 3, 4, 7, 5), s2off=None, half_c=None,
           pade_fp8=()):
    import concourse.bacc as bacc
    import concourse.mybir as mybir
    import concourse.tile as tile

    f32 = mybir.dt.float32
    f32r = mybir.dt.float32r
    bf16 = mybir.dt.bfloat16
    f8 = mybir.dt.float8e4
    Tanh = mybir.ActivationFunctionType.Tanh
    DR = mybir.MatmulPerfMode.DoubleRow
    mult = mybir.AluOpType.mult
    add = mybir.AluOpType.add

    nc = bacc.Bacc("TRN2", target_bir_lowering=False, debug=False,
                   num_devices=N_CORES)

    w1s_d = nc.dram_tensor("w1s", [128, 256], bf16, kind="ExternalInput").ap()
    w1u_d = nc.dram_tensor("w1u", [128, 512], f8, kind="ExternalInput").ap()
    w2_d = nc.dram_tensor("w2", [128, 1024], f8, kind="ExternalInput").ap()
    w2b_d = nc.dram_tensor("w2b", [128, 512], bf16, kind="ExternalInput").ap()
    w3_d = nc.dram_tensor("w3", [128, 14], bf16, kind="ExternalInput").ap()
    i103_d = nc.dram_tensor("i103", [103, 104], f32r,
                            kind="ExternalInput").ap()
    s0_d = nc.dram_tensor("s0", [256, 512], f32r, kind="ExternalInput").ap()
    ctrl_d = nc.dram_tensor("ctrl", [horizon * 8, 1024], f8,
                            kind="ExternalInput").ap()
    out_d = nc.dram_tensor("out", [horizon * 2 * 103, 512], bf16,
                           kind="ExternalOutput").ap()

    with tile.TileContext(nc) as tc:
        with (
            tc.tile_pool(name="persist", bufs=1) as pp,
            tc.tile_pool(name="sT", bufs=8) as sp_,
            tc.tile_pool(name="uT", bufs=4) as up_,
            tc.tile_pool(name="h1", bufs=5) as h1p,
            tc.tile_pool(name="h2", bufs=4) as h2p,
            tc.tile_pool(name="pxh", bufs=3) as xp_,
            tc.tile_pool(name="pscr", bufs=3) as scr,
            tc.tile_pool(name="psA", bufs=3, space="PSUM") as psA,
            tc.tile_pool(name="psZ", bufs=1, space="PSUM") as psZ,
        ):
            w1s_sb = pp.tile([128, 256], bf16, tag="w1s")
            w1u_sb = pp.tile([128, 512], f8, tag="w1u")
            w2sb = pp.tile([128, 1024], f8, tag="w2")
            w2bsb = pp.tile([128, 512], bf16, tag="w2b")
            w3sb = pp.tile([128, 14], bf16, tag="w3")
            i1sb = pp.tile([103, 104], f32r, tag="i103")
            s0sb = pp.tile([128, 1024], f32r, tag="s0")

            nc.sync.dma_start(w1s_sb[:, :], w1s_d)
            nc.sync.dma_start(w1u_sb[:, :], w1u_d)
            nc.sync.dma_start(w2sb[:, :], w2_d)
            nc.sync.dma_start(w2bsb[:, :], w2b_d)
            nc.sync.dma_start(w3sb[:, :], w3_d)
            nc.sync.dma_start(i1sb[:, :], i103_d)
            nc.sync.dma_start(s0sb[:, 0:512], s0_d[0:128, :])
            nc.sync.dma_start(s0sb[:, 512:1024], s0_d[128:256, :])

            z = [psZ.tile([103, 512], f32, tag=f"z{h}", name=f"z{h}")
                 for h in (0, 1)]
            for h in (0, 1):
                # z0 = (I/DT).T @ state0  (i103 holds eye(103)/DT)
                nc.tensor.matmul(z[h][0:103, :], i1sb[0:103, 0:103],
                                 s0sb[0:103, h * 512:(h + 1) * 512],
                                 start=True, stop=True)

            def new_uT(t):
                ut = up_.tile([128, 1024], f8, tag="uT", name=f"uT{t}")
                for q in range(4):
                    nc.sync.dma_start(
                        ut[32 * q:32 * q + 2, :],
                        ctrl_d[(t * 4 + q) * 2:(t * 4 + q) * 2 + 2, :])
                return ut

            def copy_state(t, h):
                # sT(t)[h] = z[h] * DT in bf16; doubles as the output row
                st = sp_.tile([128, 512], bf16, tag="sT", name=f"sT{t}_{h}")
                nc.vector.tensor_scalar(st[0:103, :], z[h][0:103, :],
                                        DT, None, op0=mult)
                return st

            cur_sT = {h: copy_state(0, h) for h in (0, 1)}
            cur_uT = new_uT(0)
            nxt_uT = new_uT(1) if horizon > 1 else None
            hold = {}

            def stage1(sT, uT, c):
                h, q = c // 4, c % 4
                r = 32 * q
                ph1 = psA.tile([128, 1024], f32, tag="ph1", name="ph1")
                uv = uT[r:r + NCTRL, h * 512:(h + 1) * 512].unsqueeze(
                    1).broadcast_to([NCTRL, 2, 512])
                for m in (0, 1):
                    nc.tensor.matmul(
                        ph1[:, m * 512:(m + 1) * 512],
                        w1u_sb[r:r + NCTRL, m * 256:(m + 1) * 256].rearrange(
                            "k (i m2) -> k i m2", i=2),
                        uv, start=True, stop=False, perf_mode=DR,
                        tile_position=(r, 0))
                    nc.tensor.matmul(
                        ph1[:, m * 512:(m + 1) * 512],
                        w1s_sb[r:r + NST, m * 128:(m + 1) * 128],
                        sT[h][r:r + NST, :],
                        start=False, stop=True, tile_position=(r, 0))
                if c == half_c:
                    # batch-half 0 on ACT, batch-half 1 via DVE Pade
                    h1t = h1p.tile([128, 1024], bf16, tag="h1b",
                                   name=f"h1h{c}")
                    pv = ph1[:, :].rearrange("p (m n) -> p m n", m=2)
                    hv = h1t[:, :].rearrange("p (m n) -> p m n", m=2)
                    nc.scalar.activation(hv[:, :, 0:HB], pv[:, :, 0:HB], Tanh)
                    xh = xp_.tile([128, 512], bf16, tag="xhh", name=f"xhh{c}")
                    nc.vector.tensor_copy(
                        xh[:, 0:512].rearrange("p (m n) -> p m n", n=HB),
                        pv[:, :, HB:512])
                    hold[("x", c)] = xh
                    hold[c] = h1t
                elif c in pade:
                    # tanh offloaded to DVE as x(15+t)/(15+6t), t=x^2
                    xh = xp_.tile([128, 1024], bf16, tag="xh", name=f"xh{c}")
                    nc.vector.tensor_copy(xh[:, :], ph1[:, :])
                    hold[("x", c)] = xh
                else:
                    h1t = h1p.tile([128, 1024], f8, tag="h1")
                    nc.scalar.activation(h1t[:, :], ph1[:, :], Tanh)
                    hold[c] = h1t

            def pade_sq(c, on_pool=False, w=1024):
                # t = x^2; den = 6t+15 (den on Pool when on_pool)
                xh = hold[("x", c)]
                tt_ = scr.tile([128, w], bf16, tag="tt", name=f"tt{c}")
                nc.vector.tensor_mul(tt_[:, 0:w], xh[:, 0:w], xh[:, 0:w])
                den = scr.tile([128, w], f32, tag="den", name=f"den{c}")
                eng = nc.gpsimd if on_pool else nc.vector
                eng.tensor_scalar(den[:, 0:w], tt_[:, 0:w], 6.0, 15.0,
                                  op0=mult, op1=add)
                hold[("den", c)] = den

            def pade_rcp(c, on_pool=False, w=1024):
                den = hold.pop(("den", c))
                rcp = scr.tile([128, w], f32, tag="rcp", name=f"rcp{c}")
                nc.vector.reciprocal_approx_fast(rcp[:, 0:w], den[:, 0:w])
                g = scr.tile([128, w], bf16, tag="g", name=f"g{c}")
                eng = nc.gpsimd if on_pool else nc.vector
                eng.tensor_scalar(g[:, 0:w], rcp[:, 0:w], 12.5, 1.0 / 6.0,
                                  op0=mult, op1=add)
                hold[("g", c)] = g

            def pade_fin(c):
                xh = hold.pop(("x", c))
                g = hold.pop(("g", c))
                if c in pade_fp8:
                    h1t = h1p.tile([128, 1024], f8, tag="h1", name=f"h1p{c}")
                else:
                    h1t = h1p.tile([128, 1024], bf16, tag="h1b",
                                   name=f"h1p{c}")
                nc.vector.tensor_mul(h1t[:, :], xh[:, :], g[:, :])
                hold[c] = h1t

            HB = 256  # batch-half width for the half-pade chunk

            def pade_fin_half(c):
                # write Pade result into the DVE half of h1t (batch cols
                # HB..511 of each hid half)
                xh = hold.pop(("x", c))
                g = hold.pop(("g", c))
                h1t = hold[c]
                dst = h1t[:, :].rearrange("p (m n) -> p m n", m=2)[:, :, HB:512]
                nc.vector.tensor_mul(
                    dst, xh[:, 0:512].rearrange("p (m n) -> p m n", n=HB),
                    g[:, 0:512].rearrange("p (m n) -> p m n", n=HB))

            def pade_chain(c):
                pade_sq(c)
                pade_rcp(c)
                pade_fin(c)

            def stage2(t, c):
                h, q = c // 4, c % 4
                r = 32 * q
                h1t = hold.pop(c)
                ph2 = psA.tile([128, 1024], f32, tag="ph1", name="ph2")
                if (c in pade or c == half_c) and c not in pade_fp8:
                    for m in (0, 1):
                        for i in (0, 1):
                            nc.tensor.matmul(
                                ph2[:, m * 512:(m + 1) * 512],
                                w2bsb[:, m * 256 + i * 128:
                                      m * 256 + (i + 1) * 128],
                                h1t[:, i * 512:(i + 1) * 512],
                                start=(i == 0), stop=(i == 1))
                else:
                    h1v = h1t[:, :].rearrange("k (i n) -> k i n", i=2)
                    for m in (0, 1):
                        for rphase in (0, 1):  # fp8 weight + fp8 residual
                            nc.tensor.matmul(
                                ph2[:, m * 512:(m + 1) * 512],
                                w2sb[:, m * 512 + rphase * 256:
                                     m * 512 + (rphase + 1) * 256].rearrange(
                                    "k (i m2) -> k i m2", i=2),
                                h1v, start=(rphase == 0), stop=(rphase == 1),
                                perf_mode=DR)
                h2t = h2p.tile([128, 1024], bf16, tag="h2")
                nc.scalar.activation(h2t[:, :], ph2[:, :], Tanh)
                for i in (0, 1):
                    nc.tensor.matmul(
                        z[h][r:r + NST, :],
                        w3sb[:, i * NST:(i + 1) * NST],
                        h2t[:, i * 512:(i + 1) * 512],
                        start=False, stop=(i == 1),
                        skip_group_check=True, tile_position=(0, r))

            # schedule: pade chunks (half-1 only) run stage1 first; their
            # stage2 waits for the DVE chain; half-1 stage2s complete early
            # so copy_state(h1) lands well before the next step needs it.
            assert all(c >= 4 for c in pade)
            npd = [c for c in range(NCH) if c not in pade]
            h0n = [c for c in npd if c < 4]
            h1n = [c for c in npd if c >= 4]
            if s1o is None:
                s1o = h0n[:2] + list(pade) + h0n[2:] + h1n
            if s2o is None:
                s2o = h0n + h1n + list(pade)
            s1o = list(s1o)
            s2o = list(s2o)
            assert sorted(s1o) == list(range(NCH)), s1o
            assert sorted(s2o) == list(range(NCH)), s2o
            k3 = len(pade) >= 3
            chain_at = {}
            if not k3:
                for c in pade:
                    chain_at.setdefault(s1o.index(c) + 2, []).append(c)
                if half_c is not None:
                    p = s1o.index(half_c)
                    chain_at.setdefault(p + 2, []).append(("sqh", half_c))
                    chain_at.setdefault(p + 3, []).append(("rcph", half_c))
                    chain_at.setdefault(p + 4, []).append(("finh", half_c))
            else:
                for c in pade:
                    p = s1o.index(c)
                    chain_at.setdefault(p + 2, []).append(("sq", c))
                    chain_at.setdefault(p + 4, []).append(("rcp", c))
                    chain_at.setdefault(p + 5, []).append(("fin", c))
            if s2off is None:
                s2off = 2 + len(pade) + (1 if pade else 0)
            for t in range(horizon):
                nxt_sT = {}
                done2 = set()
                n_iter = max(NCH + s2off,
                             max(chain_at) + 1 if chain_at else 0)
                npre = 0  # stage1s pre-emitted at t-1
                for i in range(n_iter):
                    if s2off <= i < NCH + s2off:
                        c2 = s2o[i - s2off]
                        stage2(t, c2)
                        done2.add(c2)
                        if done2 >= {0, 1, 2, 3} and 0 not in nxt_sT:
                            nxt_sT[0] = copy_state(t + 1, 0)
                            nc.sync.dma_start(
                                out_d[(t * 2) * 103:(t * 2) * 103 + 103, :],
                                nxt_sT[0][0:103, :])
                    if i < NCH and (t == 0 or i >= npre):
                        stage1(cur_sT, cur_uT, s1o[i])
                    for pc in chain_at.get(i, ()):
                        if isinstance(pc, tuple):
                            kind, cc = pc
                            if kind == "sq":
                                pade_sq(cc, on_pool=True)
                            elif kind == "rcp":
                                pade_rcp(cc, on_pool=True)
                            elif kind == "fin":
                                pade_fin(cc)
                            elif kind == "sqh":
                                pade_sq(cc, w=512)
                            elif kind == "rcph":
                                pade_rcp(cc, w=512)
                            else:
                                pade_fin_half(cc)
                        else:
                            pade_chain(pc)
                    # pre-emit next step's first stage1s into this tail
                    if t + 1 < horizon and i >= NCH + s2off - 1 - npre:
                        j = i - (NCH + s2off - 1 - npre)
                        if j < npre and s1o[j] < 4:
                            stage1(nxt_sT, nxt_uT, s1o[j])
                nxt_sT[1] = copy_state(t + 1, 1)
                nc.sync.dma_start(
                    out_d[(t * 2 + 1) * 103:(t * 2 + 1) * 103 + 103, :],
                    nxt_sT[1][0:103, :])
                cur_sT = nxt_sT
                cur_uT = nxt_uT
                if t + 2 < horizon:
                    nxt_uT = new_uT(t + 2)

    nc.compile()
    return nc


def _get_nc(**kw):
    key = repr(sorted(kw.items()))
    if key not in _CACHE:
        _CACHE[key] = _build(**kw)
    return _CACHE[key]


def _pack_inputs(x, W1, W2, W3):
    import ml_dtypes
    f8 = ml_dtypes.float8_e4m3
    bf16 = ml_dtypes.bfloat16

    w1s = np.zeros((128, 256), np.float32)
    w1u = np.zeros((128, 512), np.float32)
    for q in range(4):
        w1s[32 * q:32 * q + NST, :] = W1[NCTRL:F, :]
        for m in (0, 1):
            for i in (0, 1):
                w1u[32 * q:32 * q + NCTRL,
                    m * 256 + i * 128:m * 256 + (i + 1) * 128] = \
                    0.5 * W1[0:NCTRL, m * 128:(m + 1) * 128]
    def pack_res(Wfull, blk_m):
        # [128, 2*2*blk_m] fp8 A|R per m-half: (m, phase, ktile, blk_m)
        nm = Wfull.shape[1] // blk_m
        A = Wfull.astype(f8).astype(np.float32)
        R = (Wfull - A)
        outw = np.zeros((128, nm * 2 * 2 * blk_m), np.float32)
        for m in range(nm):
            for phase, Wp in ((0, A), (1, R)):
                for i in (0, 1):
                    c0 = m * 2 * 2 * blk_m + phase * 2 * blk_m + i * blk_m
                    outw[:, c0:c0 + blk_m] = \
                        Wp[i * 128:(i + 1) * 128, m * blk_m:(m + 1) * blk_m]
        return outw
    w2 = pack_res(W2, 128)
    w2b = np.zeros((128, 512), np.float32)
    for m in (0, 1):
        for i in (0, 1):
            w2b[:, m * 256 + i * 128:m * 256 + (i + 1) * 128] = \
                W2[i * 128:(i + 1) * 128, m * 128:(m + 1) * 128]
    w3 = np.zeros((128, 14), np.float32)
    for i in (0, 1):
        w3[:, i * NST:(i + 1) * NST] = W3[i * 128:(i + 1) * 128, :]
    i103 = np.zeros((103, 104), np.float32)
    i103[:, 0:103] = np.eye(103, dtype=np.float32) / DT

    # per-core tensors
    xs = x.reshape(N_CORES, B_CORE, H, F)
    s0 = np.zeros((N_CORES, 256, 512), np.float32)
    ctrl = np.zeros((N_CORES, H * 8, 1024), np.float32)
    for c in range(NCH):
        h, q = c // 4, c % 4
        blk = xs[:, c * NT:(c + 1) * NT]          # [8, 512, H, F]
        s0[:, h * 128 + 32 * q:h * 128 + 32 * q + NST, :] = \
            blk[:, :, 0, NCTRL:F].transpose(0, 2, 1)
        # ctrl rows (t, q, j), cols h*512+n
        ctrl[:, :, h * 512:(h + 1) * 512].reshape(
            N_CORES, H, 4, 2, 512)[:, :, q, :, :] = \
            blk[:, :, :, 0:NCTRL].transpose(0, 2, 3, 1)
    return {
        "w1s": w1s.astype(bf16),
        "w1u": w1u.astype(f8),
        "w2": w2.astype(f8),
        "w2b": w2b.astype(bf16),
        "w3": w3.astype(bf16),
        "i103": i103,
        "s0": s0,
        "ctrl": ctrl.astype(f8),
    }


def _unpack_output(res):
    out = np.empty((B_TOTAL, H, NST), np.float32)
    outs = out.reshape(N_CORES, NCH, NT, H, NST)
    for core in range(N_CORES):
        o = np.asarray(res.results[core]["out"]).astype(np.float32)
        o = o.reshape(H, 2, 103, 512)
        for c in range(NCH):
            h, q = c // 4, c % 4
            outs[core, c] = o[:, h, 32 * q:32 * q + NST, :].transpose(2, 0, 1)
    return out


def _run(x, W1, b1, W2, b2, W3, b3, **spmd_kwargs):
    import concourse.bass_utils as bass_utils

    x = np.ascontiguousarray(np.asarray(x, dtype=np.float32))
    W1 = np.asarray(W1, dtype=np.float32)
    W2 = np.asarray(W2, dtype=np.float32)
    W3 = np.asarray(W3, dtype=np.float32)
    for b in (b1, b2, b3):
        assert not np.any(np.asarray(b)), "kernel built for zero biases"

    nc = _get_nc()
    packed = _pack_inputs(x, W1, W2, W3)
    shared = {k: packed[k]
              for k in ("w1s", "w1u", "w2", "w2b", "w3", "i103")}
    in_maps = []
    for c in range(N_CORES):
        m = dict(shared)
        m["s0"] = packed["s0"][c]
        m["ctrl"] = packed["ctrl"][c]
        in_maps.append(m)
    res = bass_utils.run_bass_kernel_spmd(nc, in_maps,
                                          core_ids=list(range(N_CORES)),
                                          **spmd_kwargs)
    return _unpack_output(res), res


def kernel(x, W1, b1, W2, b2, W3, b3):
    out, _ = _run(x, W1, b1, W2, b2, W3, b3)
    return out


# revision 35
# speedup vs baseline: 1.0046x; 1.0046x over previous
"""Trainium2 Bass kernel for the 50-step autoregressive MLP rollout.

Per batch row b: state = x[b,0,2:9]; for t: h1 = tanh([u_t, s] @ W1);
h2 = tanh(h1 @ W2); s += DT * (h2 @ W3); out[b,t] = s.

Pure data parallel: 4096 rows/core on 8 cores; no collectives.  The
cost-model timeline is ~759us/core vs 955us for the f32r baseline; the
Activation engine (the bottleneck: 2x256 tanh per row-step at 1 elem/
lane/cycle) runs at ~96% occupancy.

Key design points:
- Feature-major on chip.  Batch in 8 chunks of 512; chunk c = (h, q),
  half h = c//4, quadrant q = c%4.
- The integrator state lives in PSUM for the whole scan as z = state/DT
  ([103, 512] f32 per half; quadrant q's 7 state rows at partitions
  32q..32q+6).  Layer 3 (bf16) accumulates into z with start=False.
  The 1/DT scaling keeps W3 at its natural scale (DT*W3 would sit in
  fp8-e4m3's denormal range; it also lets fp8 be considered for W3).
- One DVE tensor_scalar per half per step copies z*DT -> bf16 sT tile,
  which doubles as layer-1 moving operand and as the DMA'd output.
- Layer 1: controls via fp8 DoubleRow (stationary holds W1u/2 twice,
  moving broadcast stride-0 - full DR rate with K=2) + state in bf16,
  accumulated per quadrant via tile_position.
- Layer 2: fp8 DoubleRow with an fp8 *residual* correction (W2 ~=
  q8(W2) + q8(W2 - q8(W2)), two DR matmuls) - fp8 speed, ~bf16-class
  systematic error.  DoubleRow needs M>=32 and column position 0,
  which is why layer 3 (M=7, quadrant positions) stays bf16.
- Two chunks per step bypass the Activation engine for the h1 tanh:
  a DVE chain computes the Pade(3,2) approximant x(15+x^2)/(15+6x^2)
  (max err ~5e-3 on the observed |x|<=2.9 range) via tensor ops and
  reciprocal_approx_fast; their layer 2 runs bf16 on a separate W2
  copy.  This offloads ~12% of the activation-engine work.
- All PSUM psum tiles for h1/h2 preactivations come from one shared
  3-deep [128,1024] ring (6 banks) + 2 z banks = 8 banks; the 3-deep
  rotation hides the write-after-read semaphore loop that a dedicated
  single ph2 buffer would serialize on.
- stage2 (L2+tanh+L3) trails stage1 (L1+tanh) by >=2 chunks, stage2
  is emitted before stage1 within an iteration, and the s1o/s2o chunk
  orders interleave the Pade chunks so every engine handoff has >=1
  activation-slot of slack (schedules found empirically against the
  cost-model timeline).
- Host does all packing/unpacking (transposes, dtype casts, fp8/bf16
  quantization, output reshape); only the scan runs on device.  Biases
  are zeros per the spec and asserted so.
- End-to-end rel err vs the jax reference: 4.9e-3 (tolerance 2e-2).
"""

import numpy as np

B_TOTAL = 32768
N_CORES = 8
B_CORE = B_TOTAL // N_CORES      # 4096
H = 50
F = 9
NCTRL = 2
NST = 7
HID = 256
DT = 0.02
NT = 512                         # chunk batch size
NCH = B_CORE // NT               # 8 chunks

_CACHE = {}


def _build(horizon=H, pade=(4, 5), s1o=(0, 1, 2, 4, 3, 5, 6, 7),
           s2o=(0, 1, 2, 6, 3, 4, 7, 5), s2off=None, half_c=None,
           pade_fp8=()):
    import concourse.bacc as bacc
    import concourse.mybir as mybir
    import concourse.tile as tile

    f32 = mybir.dt.float32
    f32r = mybir.dt.float32r
    bf16 = mybir.dt.bfloat16
    f8 = mybir.dt.float8e4
    Tanh = mybir.ActivationFunctionType.Tanh
    DR = mybir.MatmulPerfMode.DoubleRow
    mult = mybir.AluOpType.mult
    add = mybir.AluOpType.add

    nc = bacc.Bacc("TRN2", target_bir_lowering=False, debug=False,
                   num_devices=N_CORES)

    w1s_d = nc.dram_tensor("w1s", [128, 256], bf16, kind="ExternalInput").ap()
    w1u_d = nc.dram_tensor("w1u", [128, 512], f8, kind="ExternalInput").ap()
    w2_d = nc.dram_tensor("w2", [128, 1024], f8, kind="ExternalInput").ap()
    w2b_d = nc.dram_tensor("w2b", [128, 512], bf16, kind="ExternalInput").ap()
    w3_d = nc.dram_tensor("w3", [128, 14], bf16, kind="ExternalInput").ap()
    i103_d = nc.dram_tensor("i103", [103, 104], f32r,
                            kind="ExternalInput").ap()
    s0_d = nc.dram_tensor("s0", [256, 512], f32r, kind="ExternalInput").ap()
    ctrl_d = nc.dram_tensor("ctrl", [horizon * 8, 1024], f8,
                            kind="ExternalInput").ap()
    out_d = nc.dram_tensor("out", [horizon * 2 * 103, 512], bf16,
                           kind="ExternalOutput").ap()

    with tile.TileContext(nc) as tc:
        with (
            tc.tile_pool(name="persist", bufs=1) as pp,
            tc.tile_pool(name="sT", bufs=8) as sp_,
            tc.tile_pool(name="uT", bufs=4) as up_,
            tc.tile_pool(name="h1", bufs=5) as h1p,
            tc.tile_pool(name="h2", bufs=4) as h2p,
            tc.tile_pool(name="pxh", bufs=3) as xp_,
            tc.tile_pool(name="pscr", bufs=3) as scr,
            tc.tile_pool(name="psA", bufs=3, space="PSUM") as psA,
            tc.tile_pool(name="psZ", bufs=1, space="PSUM") as psZ,
        ):
            w1s_sb = pp.tile([128, 256], bf16, tag="w1s")
            w1u_sb = pp.tile([128, 512], f8, tag="w1u")
            w2sb = pp.tile([128, 1024], f8, tag="w2")
            w2bsb = pp.tile([128, 512], bf16, tag="w2b")
            w3sb = pp.tile([128, 14], bf16, tag="w3")
            i1sb = pp.tile([103, 104], f32r, tag="i103")
            s0sb = pp.tile([128, 1024], f32r, tag="s0")

            # critical-path first: z-init needs i103+s0, then L1 needs
            # w1s/w1u, then L2/L3 weights
            nc.sync.dma_start(i1sb[:, :], i103_d)
            nc.sync.dma_start(s0sb[:, 0:512], s0_d[0:128, :])
            nc.sync.dma_start(s0sb[:, 512:1024], s0_d[128:256, :])
            nc.sync.dma_start(w1s_sb[:, :], w1s_d)
            nc.sync.dma_start(w1u_sb[:, :], w1u_d)

            z = [psZ.tile([103, 512], f32, tag=f"z{h}", name=f"z{h}")
                 for h in (0, 1)]
            for h in (0, 1):
                # z0 = (I/DT).T @ state0  (i103 holds eye(103)/DT)
                nc.tensor.matmul(z[h][0:103, :], i1sb[0:103, 0:103],
                                 s0sb[0:103, h * 512:(h + 1) * 512],
                                 start=True, stop=True)

            def new_uT(t):
                ut = up_.tile([128, 1024], f8, tag="uT", name=f"uT{t}")
                for q in range(4):
                    nc.sync.dma_start(
                        ut[32 * q:32 * q + 2, :],
                        ctrl_d[(t * 4 + q) * 2:(t * 4 + q) * 2 + 2, :])
                return ut

            def copy_state(t, h):
                # sT(t)[h] = z[h] * DT in bf16; doubles as the output row
                st = sp_.tile([128, 512], bf16, tag="sT", name=f"sT{t}_{h}")
                nc.vector.tensor_scalar(st[0:103, :], z[h][0:103, :],
                                        DT, None, op0=mult)
                return st

            cur_sT = {h: copy_state(0, h) for h in (0, 1)}
            cur_uT = new_uT(0)
            # stage2 weights are not needed until ~6us in; issue after uT(0)
            nc.sync.dma_start(w2sb[:, :], w2_d)
            nc.sync.dma_start(w2bsb[:, :], w2b_d)
            nc.sync.dma_start(w3sb[:, :], w3_d)
            nxt_uT = new_uT(1) if horizon > 1 else None
            hold = {}

            def stage1(sT, uT, c):
                h, q = c // 4, c % 4
                r = 32 * q
                ph1 = psA.tile([128, 1024], f32, tag="ph1", name="ph1")
                uv = uT[r:r + NCTRL, h * 512:(h + 1) * 512].unsqueeze(
                    1).broadcast_to([NCTRL, 2, 512])
                for m in (0, 1):
                    nc.tensor.matmul(
                        ph1[:, m * 512:(m + 1) * 512],
                        w1u_sb[r:r + NCTRL, m * 256:(m + 1) * 256].rearrange(
                            "k (i m2) -> k i m2", i=2),
                        uv, start=True, stop=False, perf_mode=DR,
                        tile_position=(r, 0))
                    nc.tensor.matmul(
                        ph1[:, m * 512:(m + 1) * 512],
                        w1s_sb[r:r + NST, m * 128:(m + 1) * 128],
                        sT[h][r:r + NST, :],
                        start=False, stop=True, tile_position=(r, 0))
                if c == half_c:
                    # batch-half 0 on ACT, batch-half 1 via DVE Pade
                    h1t = h1p.tile([128, 1024], bf16, tag="h1b",
                                   name=f"h1h{c}")
                    pv = ph1[:, :].rearrange("p (m n) -> p m n", m=2)
                    hv = h1t[:, :].rearrange("p (m n) -> p m n", m=2)
                    nc.scalar.activation(hv[:, :, 0:HB], pv[:, :, 0:HB], Tanh)
                    xh = xp_.tile([128, 512], bf16, tag="xhh", name=f"xhh{c}")
                    nc.vector.tensor_copy(
                        xh[:, 0:512].rearrange("p (m n) -> p m n", n=HB),
                        pv[:, :, HB:512])
                    hold[("x", c)] = xh
                    hold[c] = h1t
                elif c in pade:
                    # tanh offloaded to DVE as x(15+t)/(15+6t), t=x^2
                    xh = xp_.tile([128, 1024], bf16, tag="xh", name=f"xh{c}")
                    nc.vector.tensor_copy(xh[:, :], ph1[:, :])
                    hold[("x", c)] = xh
                else:
                    h1t = h1p.tile([128, 1024], f8, tag="h1")
                    nc.scalar.activation(h1t[:, :], ph1[:, :], Tanh)
                    hold[c] = h1t

            def pade_sq(c, on_pool=False, w=1024):
                # t = x^2; den = 6t+15 (den on Pool when on_pool)
                xh = hold[("x", c)]
                tt_ = scr.tile([128, w], bf16, tag="tt", name=f"tt{c}")
                nc.vector.tensor_mul(tt_[:, 0:w], xh[:, 0:w], xh[:, 0:w])
                den = scr.tile([128, w], f32, tag="den", name=f"den{c}")
                eng = nc.gpsimd if on_pool else nc.vector
                eng.tensor_scalar(den[:, 0:w], tt_[:, 0:w], 6.0, 15.0,
                                  op0=mult, op1=add)
                hold[("den", c)] = den

            def pade_rcp(c, on_pool=False, w=1024):
                den = hold.pop(("den", c))
                rcp = scr.tile([128, w], f32, tag="rcp", name=f"rcp{c}")
                nc.vector.reciprocal_approx_fast(rcp[:, 0:w], den[:, 0:w])
                g = scr.tile([128, w], bf16, tag="g", name=f"g{c}")
                eng = nc.gpsimd if on_pool else nc.vector
                eng.tensor_scalar(g[:, 0:w], rcp[:, 0:w], 12.5, 1.0 / 6.0,
                                  op0=mult, op1=add)
                hold[("g", c)] = g

            def pade_fin(c):
                xh = hold.pop(("x", c))
                g = hold.pop(("g", c))
                if c in pade_fp8:
                    h1t = h1p.tile([128, 1024], f8, tag="h1", name=f"h1p{c}")
                else:
                    h1t = h1p.tile([128, 1024], bf16, tag="h1b",
                                   name=f"h1p{c}")
                nc.vector.tensor_mul(h1t[:, :], xh[:, :], g[:, :])
                hold[c] = h1t

            HB = 256  # batch-half width for the half-pade chunk

            def pade_fin_half(c):
                # write Pade result into the DVE half of h1t (batch cols
                # HB..511 of each hid half)
                xh = hold.pop(("x", c))
                g = hold.pop(("g", c))
                h1t = hold[c]
                dst = h1t[:, :].rearrange("p (m n) -> p m n", m=2)[:, :, HB:512]
                nc.vector.tensor_mul(
                    dst, xh[:, 0:512].rearrange("p (m n) -> p m n", n=HB),
                    g[:, 0:512].rearrange("p (m n) -> p m n", n=HB))

            def pade_chain(c):
                pade_sq(c)
                pade_rcp(c)
                pade_fin(c)

            def stage2(t, c):
                h, q = c // 4, c % 4
                r = 32 * q
                h1t = hold.pop(c)
                ph2 = psA.tile([128, 1024], f32, tag="ph1", name="ph2")
                if (c in pade or c == half_c) and c not in pade_fp8:
                    for m in (0, 1):
                        for i in (0, 1):
                            nc.tensor.matmul(
                                ph2[:, m * 512:(m + 1) * 512],
                                w2bsb[:, m * 256 + i * 128:
                                      m * 256 + (i + 1) * 128],
                                h1t[:, i * 512:(i + 1) * 512],
                                start=(i == 0), stop=(i == 1))
                else:
                    h1v = h1t[:, :].rearrange("k (i n) -> k i n", i=2)
                    for m in (0, 1):
                        for rphase in (0, 1):  # fp8 weight + fp8 residual
                            nc.tensor.matmul(
                                ph2[:, m * 512:(m + 1) * 512],
                                w2sb[:, m * 512 + rphase * 256:
                                     m * 512 + (rphase + 1) * 256].rearrange(
                                    "k (i m2) -> k i m2", i=2),
                                h1v, start=(rphase == 0), stop=(rphase == 1),
                                perf_mode=DR)
                h2t = h2p.tile([128, 1024], bf16, tag="h2")
                nc.scalar.activation(h2t[:, :], ph2[:, :], Tanh)
                for i in (0, 1):
                    nc.tensor.matmul(
                        z[h][r:r + NST, :],
                        w3sb[:, i * NST:(i + 1) * NST],
                        h2t[:, i * 512:(i + 1) * 512],
                        start=False, stop=(i == 1),
                        skip_group_check=True, tile_position=(0, r))

            # schedule: pade chunks (half-1 only) run stage1 first; their
            # stage2 waits for the DVE chain; half-1 stage2s complete early
            # so copy_state(h1) lands well before the next step needs it.
            assert all(c >= 4 for c in pade)
            npd = [c for c in range(NCH) if c not in pade]
            h0n = [c for c in npd if c < 4]
            h1n = [c for c in npd if c >= 4]
            if s1o is None:
                s1o = h0n[:2] + list(pade) + h0n[2:] + h1n
            if s2o is None:
                s2o = h0n + h1n + list(pade)
            s1o = list(s1o)
            s2o = list(s2o)
            assert sorted(s1o) == list(range(NCH)), s1o
            assert sorted(s2o) == list(range(NCH)), s2o
            k3 = len(pade) >= 3
            chain_at = {}
            if not k3:
                for c in pade:
                    chain_at.setdefault(s1o.index(c) + 2, []).append(c)
                if half_c is not None:
                    p = s1o.index(half_c)
                    chain_at.setdefault(p + 2, []).append(("sqh", half_c))
                    chain_at.setdefault(p + 3, []).append(("rcph", half_c))
                    chain_at.setdefault(p + 4, []).append(("finh", half_c))
            else:
                for c in pade:
                    p = s1o.index(c)
                    chain_at.setdefault(p + 2, []).append(("sq", c))
                    chain_at.setdefault(p + 4, []).append(("rcp", c))
                    chain_at.setdefault(p + 5, []).append(("fin", c))
            if s2off is None:
                s2off = 2 + len(pade) + (1 if pade else 0)
            for t in range(horizon):
                nxt_sT = {}
                done2 = set()
                n_iter = max(NCH + s2off,
                             max(chain_at) + 1 if chain_at else 0)
                npre = 0  # stage1s pre-emitted at t-1
                for i in range(n_iter):
                    if s2off <= i < NCH + s2off:
                        c2 = s2o[i - s2off]
                        stage2(t, c2)
                        done2.add(c2)
                        if done2 >= {0, 1, 2, 3} and 0 not in nxt_sT:
                            nxt_sT[0] = copy_state(t + 1, 0)
                            nc.sync.dma_start(
                                out_d[(t * 2) * 103:(t * 2) * 103 + 103, :],
                                nxt_sT[0][0:103, :])
                    if i < NCH and (t == 0 or i >= npre):
                        stage1(cur_sT, cur_uT, s1o[i])
                    for pc in chain_at.get(i, ()):
                        if isinstance(pc, tuple):
                            kind, cc = pc
                            if kind == "sq":
                                pade_sq(cc, on_pool=True)
                            elif kind == "rcp":
                                pade_rcp(cc, on_pool=True)
                            elif kind == "fin":
                                pade_fin(cc)
                            elif kind == "sqh":
                                pade_sq(cc, w=512)
                            elif kind == "rcph":
                                pade_rcp(cc, w=512)
                            else:
                                pade_fin_half(cc)
                        else:
                            pade_chain(pc)
                    # pre-emit next step's first stage1s into this tail
                    if t + 1 < horizon and i >= NCH + s2off - 1 - npre:
                        j = i - (NCH + s2off - 1 - npre)
                        if j < npre and s1o[j] < 4:
                            stage1(nxt_sT, nxt_uT, s1o[j])
                nxt_sT[1] = copy_state(t + 1, 1)
                nc.sync.dma_start(
                    out_d[(t * 2 + 1) * 103:(t * 2 + 1) * 103 + 103, :],
                    nxt_sT[1][0:103, :])
                cur_sT = nxt_sT
                cur_uT = nxt_uT
                if t + 2 < horizon:
                    nxt_uT = new_uT(t + 2)

    nc.compile()
    return nc


def _get_nc(**kw):
    key = repr(sorted(kw.items()))
    if key not in _CACHE:
        _CACHE[key] = _build(**kw)
    return _CACHE[key]


def _pack_inputs(x, W1, W2, W3):
    import ml_dtypes
    f8 = ml_dtypes.float8_e4m3
    bf16 = ml_dtypes.bfloat16

    w1s = np.zeros((128, 256), np.float32)
    w1u = np.zeros((128, 512), np.float32)
    for q in range(4):
        w1s[32 * q:32 * q + NST, :] = W1[NCTRL:F, :]
        for m in (0, 1):
            for i in (0, 1):
                w1u[32 * q:32 * q + NCTRL,
                    m * 256 + i * 128:m * 256 + (i + 1) * 128] = \
                    0.5 * W1[0:NCTRL, m * 128:(m + 1) * 128]
    def pack_res(Wfull, blk_m):
        # [128, 2*2*blk_m] fp8 A|R per m-half: (m, phase, ktile, blk_m)
        nm = Wfull.shape[1] // blk_m
        A = Wfull.astype(f8).astype(np.float32)
        R = (Wfull - A)
        outw = np.zeros((128, nm * 2 * 2 * blk_m), np.float32)
        for m in range(nm):
            for phase, Wp in ((0, A), (1, R)):
                for i in (0, 1):
                    c0 = m * 2 * 2 * blk_m + phase * 2 * blk_m + i * blk_m
                    outw[:, c0:c0 + blk_m] = \
                        Wp[i * 128:(i + 1) * 128, m * blk_m:(m + 1) * blk_m]
        return outw
    w2 = pack_res(W2, 128)
    w2b = np.zeros((128, 512), np.float32)
    for m in (0, 1):
        for i in (0, 1):
            w2b[:, m * 256 + i * 128:m * 256 + (i + 1) * 128] = \
                W2[i * 128:(i + 1) * 128, m * 128:(m + 1) * 128]
    w3 = np.zeros((128, 14), np.float32)
    for i in (0, 1):
        w3[:, i * NST:(i + 1) * NST] = W3[i * 128:(i + 1) * 128, :]
    i103 = np.zeros((103, 104), np.float32)
    i103[:, 0:103] = np.eye(103, dtype=np.float32) / DT

    # per-core tensors
    xs = x.reshape(N_CORES, B_CORE, H, F)
    s0 = np.zeros((N_CORES, 256, 512), np.float32)
    ctrl = np.zeros((N_CORES, H * 8, 1024), np.float32)
    for c in range(NCH):
        h, q = c // 4, c % 4
        blk = xs[:, c * NT:(c + 1) * NT]          # [8, 512, H, F]
        s0[:, h * 128 + 32 * q:h * 128 + 32 * q + NST, :] = \
            blk[:, :, 0, NCTRL:F].transpose(0, 2, 1)
        # ctrl rows (t, q, j), cols h*512+n
        ctrl[:, :, h * 512:(h + 1) * 512].reshape(
            N_CORES, H, 4, 2, 512)[:, :, q, :, :] = \
            blk[:, :, :, 0:NCTRL].transpose(0, 2, 3, 1)
    return {
        "w1s": w1s.astype(bf16),
        "w1u": w1u.astype(f8),
        "w2": w2.astype(f8),
        "w2b": w2b.astype(bf16),
        "w3": w3.astype(bf16),
        "i103": i103,
        "s0": s0,
        "ctrl": ctrl.astype(f8),
    }


def _unpack_output(res):
    out = np.empty((B_TOTAL, H, NST), np.float32)
    outs = out.reshape(N_CORES, NCH, NT, H, NST)
    for core in range(N_CORES):
        o = np.asarray(res.results[core]["out"]).astype(np.float32)
        o = o.reshape(H, 2, 103, 512)
        for c in range(NCH):
            h, q = c // 4, c % 4
            outs[core, c] = o[:, h, 32 * q:32 * q + NST, :].transpose(2, 0, 1)
    return out


def _run(x, W1, b1, W2, b2, W3, b3, **spmd_kwargs):
    import concourse.bass_utils as bass_utils

    x = np.ascontiguousarray(np.asarray(x, dtype=np.float32))
    W1 = np.asarray(W1, dtype=np.float32)
    W2 = np.asarray(W2, dtype=np.float32)
    W3 = np.asarray(W3, dtype=np.float32)
    for b in (b1, b2, b3):
        assert not np.any(np.asarray(b)), "kernel built for zero biases"

    nc = _get_nc()
    packed = _pack_inputs(x, W1, W2, W3)
    shared = {k: packed[k]
              for k in ("w1s", "w1u", "w2", "w2b", "w3", "i103")}
    in_maps = []
    for c in range(N_CORES):
        m = dict(shared)
        m["s0"] = packed["s0"][c]
        m["ctrl"] = packed["ctrl"][c]
        in_maps.append(m)
    res = bass_utils.run_bass_kernel_spmd(nc, in_maps,
                                          core_ids=list(range(N_CORES)),
                                          **spmd_kwargs)
    return _unpack_output(res), res


def kernel(x, W1, b1, W2, b2, W3, b3):
    out, _ = _run(x, W1, b1, W2, b2, W3, b3)
    return out
